# revision 45
# baseline (speedup 1.0000x reference)
"""Trainium2 Bass kernel for nn_CNN_88098369175825 (spiking CNN/SLSTM), v2.

Model (see reference): eeg = x[:, 0, 1:65, :] -> [B, 64, 2048]; time is chunked
into 8 pieces of 256 (t = tau*256 + s); emb = w_emb @ eeg -> LIF over tau ->
spiking LSTM over tau (per-(sample, s) token) -> mean over s -> classifier ->
LIF over tau -> mean spikes.

Sharding: pure data parallel, B=256 -> 8 cores x 32 samples.

Per-core layout: 3 "sample tiles" g of 12/12/8 samples. Within a tile, sample
slot j lives in partition rows rb(j)..rb(j)+9 where rb(j) = 10j (j<6) else
64+10(j-6); rows 60..63 / 124..127 are padding (H/SQ row 60 = +1 const).

v2 restructure vs baseline (113.8us):
- all eeg DMA (SWDGE fp32->bf16 cast) issued upfront into resident SBUF tiles
  (first two chunks small for fast pipeline fill), so the DMA engines stream
  continuously instead of being paced by compute;
- X spikes are 0/1 from a DVE is_ge (LX = bf16(w_ih) full weights; exactly the
  same effective network since 2*bf16(w/2) == bf16(w)); no ScalarE X-Sign;
- gates land in two PSUM tiles IG=[128,2,768], FO=[128,2,768] so the four
  per-gate Signs fuse into two wide ScalarE Signs (the ~450ns ACT fixed cost
  amortizes); chunked 512/256 per gate to respect PSUM bank boundaries;
- LIF work is split across engines: u-add + X01 on DVE, om = 0.2-0.2*X01 as a
  ScalarE affine activation, w-update STT on GpSimd;
- classifier matmul + final-LIF step run per-tau (pipelined into the loop) on
  GpSimd instead of as a serial tail;
- software pipeline: X-part gate matmuls and the whole emb/LIF chain for step
  t+1 are emitted to run inside step t's window; PE stream stays dense so the
  HAM clock gate holds 2.4 GHz.
"""

import numpy as np

VTH = 0.2
DECAY = 0.2
BIG = 512.0
NTAU = 8
NS = 256  # s positions per chunk
NPAIR = 16

# packed constant layout (bytes per partition)
NF32 = 8   # fp32 per-row: vthr, brow, -VTH, +0.2, hfix, bmv, +0.1, hrow2
OFF_BF = 4 * NF32            # bf16 section
NBF = 192 + 512 + 512 + 24 + 128   # Le, LX, LH, LC, Lb
CBYTES = OFF_BF + 2 * NBF

_CACHE = {}


def _rb(j):
    return 10 * j if j < 6 else 64 + 10 * (j - 6)


def _bf16(a):
    import ml_dtypes
    return np.asarray(a, np.float32).astype(ml_dtypes.bfloat16)


def build_consts(w_emb, b_emb, w_ih, w_hh, b_ih, b_hh, w_cls, b_cls):
    """Host-side constant construction. Returns one packed uint8 [128, CBYTES]."""
    f8 = np.float64

    # --- emb lhsT variants, bf16 [128, 3*64] ---
    Le = np.zeros((128, 3, 64), np.float32)
    for v in range(3):
        Le[0:64, v, 20 * v:20 * v + 10] = w_emb.T          # even sample of pair
        Le[64:128, v, 20 * v + 10:20 * v + 20] = w_emb.T   # odd sample of pair
    Le = Le.reshape(128, 192)

    # quantized weights (these define the effective network).
    # X spikes are 0/1 (DVE is_ge) so LX = bf16(w_ih) full (exact: equals
    # 2*bf16(w_ih/2)); H spikes are +-1 so LH = bf16(w_hh/2) with +sum in C.
    wih_b = _bf16(np.asarray(w_ih, f8)).astype(f8)         # [40, 10] full
    whh_q = _bf16(np.asarray(w_hh, f8) / 2.0).astype(f8)   # [40, 10]
    wcls_q = _bf16(np.asarray(w_cls, f8) / 512.0).astype(f8)

    C = (np.asarray(b_ih, f8) + np.asarray(b_hh, f8) + whh_q.sum(1))  # [40]

    LX = np.zeros((4, 128, 128), f8)
    LH = np.zeros((4, 128, 128), f8)
    for gi in range(4):
        for j in range(12):
            r = _rb(j)
            LX[gi, r:r + 10, r:r + 10] = wih_b[gi * 10:gi * 10 + 10, :].T
            LH[gi, r:r + 10, r:r + 10] = whh_q[gi * 10:gi * 10 + 10, :].T
            LH[gi, 60, r:r + 10] = C[gi * 10:gi * 10 + 10]
        # keeps gate row60 = BIG >= vth every step (H row 60 = +1)
        LH[gi, 60, 60] = BIG

    LC = np.zeros((128, 24), f8)
    for j in range(12):
        r = _rb(j)
        for o in range(2):
            LC[r:r + 10, 2 * j + o] = wcls_q[o, :]
            LC[60, 2 * j + o] = wcls_q[o, :].sum() + np.asarray(b_cls, f8)[o] / 256.0

    # fp32 per-row consts
    brow_v = np.zeros(128, f8)
    for j in range(12):
        r = _rb(j)
        brow_v[r:r + 10] = np.asarray(b_emb, f8)

    # emb bias lhsT row (K=1): partition 0 holds brow for all 128 out rows
    Lb = np.zeros((128, 128), f8)
    Lb[0, :] = brow_v

    cst_bf = np.concatenate(
        [Le.astype(f8),
         LX.transpose(1, 0, 2).reshape(128, 512),
         LH.transpose(1, 0, 2).reshape(128, 512),
         LC, Lb], axis=1)                                  # [128, NBF]
    cst_bf = _bf16(cst_bf)
    hfix = np.ones(128, f8)
    hfix[60] = -1.0
    hrow2 = np.zeros(128, f8)
    hrow2[60] = 2.0
    cst_f32 = np.stack([VTH - brow_v, brow_v, np.full(128, -VTH),
                        np.full(128, DECAY), hfix, brow_v - VTH,
                        np.full(128, 0.1), hrow2], 1).astype(np.float32)

    cst = np.zeros((128, CBYTES), np.uint8)
    cst[:, 0:OFF_BF] = cst_f32.view(np.uint8)
    cst[:, OFF_BF:CBYTES] = cst_bf.view(np.uint8)
    return dict(cst=np.ascontiguousarray(cst))


def build_nc(dbg=False, has_bias=True):
    """Builds the Bass program (identical for all cores)."""
    from concourse import bacc, mybir
    from concourse import tile

    dt = mybir.dt
    Alu = mybir.AluOpType
    Act = mybir.ActivationFunctionType

    nc = bacc.Bacc(trn_type="TRN2", target_bir_lowering=False, debug=False)
    xe = nc.dram_tensor("xe", [32, 64, 2048], dt.float32, kind="ExternalInput").ap()
    cst = nc.dram_tensor("cst", [128, CBYTES], dt.uint8, kind="ExternalInput").ap()
    out_d = nc.dram_tensor("out", [32, 2], dt.float32, kind="ExternalOutput").ap()
    dbg_pool = nc.dram_tensor("dbg_pool", [128, 24], dt.float32, kind="ExternalOutput").ap()
    dbg_log = nc.dram_tensor("dbg_log", [24, 24], dt.float32, kind="ExternalOutput").ap()
    if dbg:
        dbg_X = nc.dram_tensor("dbg_X", [128, 8, 3, 256], dt.bfloat16, kind="ExternalOutput").ap()
        dbg_H = nc.dram_tensor("dbg_H", [128, 8, 768], dt.bfloat16, kind="ExternalOutput").ap()

    # which (variant) pairs feed each (tile g, group q)
    def group_pairs(g, q):
        if g < 2:
            return [(6 * g + 3 * q + v, v) for v in range(3)]
        return [(12 + v, v) for v in range(3)] if q == 0 else [(15, 0)]

    with tile.TileContext(nc) as tc:
        with tc.tile_pool(name="const", bufs=1) as cpool, \
             tc.tile_pool(name="eeg", bufs=1) as epool, \
             tc.tile_pool(name="state", bufs=1) as spool, \
             tc.tile_pool(name="work", bufs=2) as wpool, \
             tc.tile_pool(name="psIG", bufs=1, space="PSUM") as psIG, \
             tc.tile_pool(name="psFO", bufs=1, space="PSUM") as psFO, \
             tc.tile_pool(name="psEC", bufs=1, space="PSUM") as psEC:

            c_t = cpool.tile([128, CBYTES], dt.uint8, tag="cst")
            nc.sync.dma_start(c_t[:], cst)
            cf_t = c_t[:, 0:OFF_BF].bitcast(dt.float32)         # [128, 2]
            cb_t = c_t[:, OFF_BF:CBYTES].bitcast(dt.bfloat16)   # [128, NBF]

            vthr = cf_t[:, 0:1]     # VTH - b_emb (per row)
            brow = cf_t[:, 1:2]     # b_emb (per row)
            nvth = cf_t[:, 2:3]     # -VTH
            p2 = cf_t[:, 3:4]       # +0.2
            hfix = cf_t[:, 4:5]     # +1 (-1 on row 60)
            bmv = cf_t[:, 5:6]      # b_emb - VTH (per row)
            p1 = cf_t[:, 6:7]       # +0.1
            hrow2 = cf_t[:, 7:8]    # 0 (+2 on row 60)
            Le = [cb_t[:, 64 * v:64 * (v + 1)] for v in range(3)]
            LX = [cb_t[:, 192 + 128 * i:192 + 128 * (i + 1)] for i in range(4)]
            LH = [cb_t[:, 704 + 128 * i:704 + 128 * (i + 1)] for i in range(4)]
            LC = cb_t[:, 1216:1240]
            Lb = cb_t[0:1, 1240:1368]

            # ---- eeg: all of it, resident in SBUF, DMA'd upfront ----
            # one 256-col chunk per tau step: emb(t) waits only its own 2MB,
            # so a DMA-starved emb matmul stalls the in-order PE queue as
            # briefly as possible; the t=0 chunk is further split by
            # pair-halves so the first emb matmuls start after ~1MB
            xsrc = xe.rearrange("(p s) c t -> (s c) p t", s=2)
            e_chunks = [(256 * k, 256) for k in range(8)]
            eeg_t = []
            for ci, (c0, w) in enumerate(e_chunks):
                et = epool.tile([128, NPAIR, w], dt.bfloat16, tag=f"eeg{ci}",
                                name=f"eeg{ci}")
                splits = {0: 4, 1: 2}.get(ci, 1)
                pstep = NPAIR // splits
                for si in range(splits):
                    p0 = si * pstep
                    nc.gpsimd.dma_start(et[:, p0:p0 + pstep, :],
                                        xsrc[:, p0:p0 + pstep, c0:c0 + w])
                eeg_t.append(et)

            def eeg_slice(t, p):
                """[128, 256] bf16 view of pair p, tau t."""
                c = 256 * t
                for (c0, w), et in zip(e_chunks, eeg_t):
                    if c0 <= c < c0 + w:
                        off = c - c0
                        return et[:, p, off:off + 256]
                raise AssertionError

            # ---- persistent state ----
            w_lif = spool.tile([128, 3, NS], dt.float32, tag="wlif")
            H = [spool.tile([128, 768], dt.bfloat16, tag=f"H{i}", name=f"H{i}")
                 for i in range(2)]
            SQ = [spool.tile([128, 768], dt.bfloat16, tag=f"SQ{i}", name=f"SQ{i}")
                  for i in range(2)]
            pooled = spool.tile([128, NTAU, 3], dt.float32, tag="pooled")
            poolbf = spool.tile([128, NTAU, 3], dt.bfloat16, tag="poolbf")
            sall = spool.tile([24, 3, NTAU], dt.float32, tag="sall")
            wfin = spool.tile([24, 3], dt.float32, tag="wfin")

            ones_t = spool.tile([1, 512], dt.bfloat16, tag="ones")
            nc.vector.memset(ones_t[:], 1.0)
            nc.vector.memset(w_lif[:], 0.0)
            nc.vector.memset(wfin[:], 0.0)
            # t=0 inputs are H[1]/SQ[1]: -1 everywhere, +1 on row 60
            for st in (H[1], SQ[1]):
                nc.vector.memset(st[:], -1.0)
                nc.vector.tensor_scalar(st[:], st[:], hfix, None,
                                        mybir.AluOpType.mult)

            # ---- PSUM tiles (8 banks exactly) ----
            IG = psIG.tile([128, 2, 768], dt.float32, tag="IG")   # 3 banks
            FO = psFO.tile([128, 2, 768], dt.float32, tag="FO")   # 3 banks
            EC = psEC.tile([128, 1024], dt.float32, tag="EC")     # 2 banks
            emb_ps = EC[:, 0:768].rearrange("p (g s) -> p g s", g=3)
            cls_ps = EC[0:24, 768:792].rearrange("p (t g) -> p t g", t=8)

            # Bank-safe chunks within a [128, 2, 768] fp32 tile (bank = 512
            # f32). Gate A: [0:512](b0), [512:768](b1). Gate B: [768:1024]
            # (b1!), [1024:1536](b2). Concurrent PSUM accumulation groups may
            # not share a bank, so gate B's first chunk cannot be X-prefilled
            # while gate A's b1 group is open — it runs X+H back-to-back at
            # H time instead.
            def mm_gates_x(ps, gis, Xf, Lset):
                """X-part prefill (start, no stop) for the prefillable chunks."""
                flat = ps.rearrange("p a b -> p (a b)")
                for (o0, wd, c0) in ((0, 512, 0), (512, 256, 512)):
                    nc.tensor.matmul(flat[:, o0:o0 + wd], Lset[gis[0]],
                                     Xf[:, c0:c0 + wd], start=True, stop=False)
                nc.tensor.matmul(flat[:, 1024:1536], Lset[gis[1]],
                                 Xf[:, 256:768], start=True, stop=False)

            def mm_gates_h(ps, gis, Xf, Hin, Lset):
                """H-part (stop) + the deferred X chunk of gate B."""
                flat = ps.rearrange("p a b -> p (a b)")
                for (o0, wd, c0) in ((0, 512, 0), (512, 256, 512)):
                    nc.tensor.matmul(flat[:, o0:o0 + wd], LH[gis[0]],
                                     Hin[:, c0:c0 + wd], start=False, stop=True)
                nc.tensor.matmul(flat[:, 768:1024], Lset[gis[1]],
                                 Xf[:, 0:256], start=True, stop=False)
                nc.tensor.matmul(flat[:, 768:1024], LH[gis[1]],
                                 Hin[:, 0:256], start=False, stop=True)
                nc.tensor.matmul(flat[:, 1024:1536], LH[gis[1]],
                                 Hin[:, 256:768], start=False, stop=True)

            def mm_emb(t):
                # per-(g, q) group: optional bias row first (K=1 matmul),
                # then the pair matmuls accumulate, last one stops — groups
                # stay sequential so no PSUM zero-region conflicts.
                for g in range(3):
                    for q in range(2):
                        o_ap = emb_ps[64 * q:64 * q + 64, g, :]
                        if has_bias:
                            nc.tensor.matmul(o_ap, Lb[:, 64 * q:64 * q + 64],
                                             ones_t[:, 0:256],
                                             start=True, stop=False)
                        plist = group_pairs(g, q)
                        for k, (p, v) in enumerate(plist):
                            nc.tensor.matmul(
                                o_ap, Le[v], eeg_slice(t, p),
                                start=(not has_bias and k == 0),
                                stop=(k == len(plist) - 1))

            def lif_u_x(t, u, X01):
                """u = w + emb (DVE); X01 = (u >= VTH) 0/1 (DVE)."""
                nc.vector.tensor_tensor(u[:], w_lif[:], emb_ps[:, :, :], Alu.add)
                nc.vector.tensor_scalar(X01[:], u[:], VTH, None, Alu.is_ge)

            def lif_w_om(t, X01, om):
                """om = 0.2 - 0.2*X01 (ScalarE)."""
                nc.scalar.activation(om[:], X01[:], Act.Identity,
                                     bias=p2, scale=-0.2)

            def lif_w_upd(t, u, om):
                """w' = v * om. On DVE: GpSimd tensor ops share SBUF ports
                with the DVE and slow concurrent DVE ops ~4x, so GpSimd is
                kept to DMA descriptor generation only."""
                nc.vector.tensor_tensor(w_lif[:], u[:], om[:], Alu.mult)

            GI_IG = (0, 2)   # gates I, G
            GI_FO = (1, 3)   # gates F, O

            # ---- prologue: LIF for t=0, X-parts for t=0, emb for t=1 ----
            u_c = wpool.tile([128, 3, NS], dt.float32, tag="u", name="u_p")
            X_c = wpool.tile([128, 3, NS], dt.bfloat16, tag="X", name="X_p")
            om_c = wpool.tile([128, 3, NS], dt.float32, tag="om", name="om_p")
            mm_emb(0)
            lif_u_x(0, u_c, X_c)
            lif_w_om(0, X_c, om_c)
            lif_w_upd(0, u_c, om_c)
            Xf_c = X_c.rearrange("p a b -> p (a b)")
            mm_gates_x(IG, GI_IG, Xf_c, LX)
            mm_gates_x(FO, GI_FO, Xf_c, LX)
            # step-0 H-parts hoisted ahead of the e1-DMA-dependent emb(1)
            # matmuls so the in-order PE queue can't stall them
            mm_gates_h(IG, GI_IG, Xf_c, H[1][:], LX)
            mm_gates_h(FO, GI_FO, Xf_c, H[1][:], LX)
            mm_emb(1)

            def emit_tail(t):
                """Final-LIF step t on logits (tiny [24,3] ops); the wfin
                update is emitted separately (later in the DVE stream)."""
                u_f = wpool.tile([24, 3], dt.float32, tag="uf", name=f"uf{t}")
                om_f = wpool.tile([24, 3], dt.float32, tag="omf", name=f"omf{t}")
                nc.vector.tensor_tensor(u_f[:], wfin[:], cls_ps[:, t, :], Alu.add)
                nc.vector.tensor_scalar(sall[:, :, t:t + 1], u_f[:], VTH, None,
                                        Alu.is_ge)
                # om_f = 0.2 - 0.2*s01 on ScalarE
                nc.scalar.activation(om_f[:], sall[:, :, t], Act.Identity,
                                     bias=p2[0:24, :], scale=-0.2)
                return u_f, om_f

            for t in range(NTAU):
                Hin, Hout = H[(t + 1) % 2], H[t % 2]
                SQin, SQout = SQ[(t + 1) % 2], SQ[t % 2]
                Xf_t = X_c.rearrange("p a b -> p (a b)")


                # PE: H-part gate matmuls for step t (+ deferred X chunks;
                # t=0's were emitted in the prologue), then the classifier
                # matmul for t-1 (its poolbf is long done, so the PE stream
                # never stalls on it), then the final-LIF step t-1
                if t > 0:
                    mm_gates_h(IG, GI_IG, Xf_t, Hin[:], LX)
                    mm_gates_h(FO, GI_FO, Xf_t, Hin[:], LX)
                # heads: LIF for t+1 — inputs ready at iteration start
                if t + 1 < NTAU:
                    u_n = wpool.tile([128, 3, NS], dt.float32, tag="u",
                                     name=f"u{t + 1}")
                    X_n = wpool.tile([128, 3, NS], dt.bfloat16, tag="X",
                                     name=f"X{t + 1}")
                    om_n = wpool.tile([128, 3, NS], dt.float32, tag="om",
                                      name=f"om{t + 1}")
                    lif_u_x(t + 1, u_n, X_n)
                    lif_w_om(t + 1, X_n, om_n)

                tail_h = None
                if t > 0:
                    nc.tensor.matmul(cls_ps[:, t - 1, :], LC,
                                     poolbf[:, t - 1, :], start=True, stop=True)
                    tail_h = emit_tail(t - 1)
                # DVE staircase: w' (om ~1.9us in), wfin (om_f ~2.2us), then
                # sig (Sign_IG ~2.6us) — each input just-in-time
                if t + 1 < NTAU:
                    lif_w_upd(t + 1, u_n, om_n)
                if tail_h is not None:
                    nc.vector.tensor_tensor(wfin[:], tail_h[0][:], tail_h[1][:],
                                            Alu.mult)

                # ScalarE: fused Signs (gate - VTH >= 0 -> +1)
                sIG = wpool.tile([128, 2, 768], dt.bfloat16, tag="sIG",
                                 name=f"sIG{t}")
                sFO = wpool.tile([128, 2, 768], dt.bfloat16, tag="sFO",
                                 name=f"sFO{t}")
                nc.scalar.activation(sIG[:], IG[:], Act.Sign, bias=nvth)
                nc.scalar.activation(sFO[:], FO[:], Act.Sign, bias=nvth)

                # DVE: sig=min(sI,sG); t2=min(sf,SQin); SQ'=max(t2,sig);
                # H'=min(so,SQ'), fused with the s-pooling (STT accum_out)
                sig = wpool.tile([128, 768], dt.bfloat16, tag="sig",
                                 name=f"sig{t}")
                t2 = wpool.tile([128, 768], dt.bfloat16, tag="t2",
                                name=f"t2{t}")
                nc.vector.tensor_tensor(sig[:], sIG[:, 0, :], sIG[:, 1, :],
                                        Alu.min)
                nc.vector.tensor_tensor(t2[:], sFO[:, 0, :], SQin[:], Alu.min)
                nc.vector.tensor_tensor(SQout[:], t2[:], sig[:], Alu.max)
                for g in range(3):
                    nc.vector.scalar_tensor_tensor(
                        Hout[:, 256 * g:256 * (g + 1)],
                        sFO[:, 1, 256 * g:256 * (g + 1)], 0.0,
                        SQout[:, 256 * g:256 * (g + 1)],
                        Alu.bypass, Alu.min,
                        accum_out=pooled[:, t, g:g + 1])
                if dbg:
                    nc.sync.dma_start(dbg_X[:, t, :, :], X_c[:])
                    nc.sync.dma_start(dbg_H[:, t, :], Hout[:])

                # PE: emb for t+2, then X-part matmuls for t+1
                if t + 1 < NTAU:
                    if t + 2 < NTAU:
                        mm_emb(t + 2)
                    Xf_n = X_n.rearrange("p a b -> p (a b)")
                    mm_gates_x(IG, GI_IG, Xf_n, LX)
                    mm_gates_x(FO, GI_FO, Xf_n, LX)
                    u_c, X_c, om_c = u_n, X_n, om_n

                # DVE: bf16 copy of pooled for the next iteration's cls matmul
                nc.vector.tensor_copy(poolbf[:, t, :], pooled[:, t, :])


            nc.tensor.matmul(cls_ps[:, NTAU - 1, :], LC,
                             poolbf[:, NTAU - 1, :], start=True, stop=True)
            emit_tail(NTAU - 1)  # wfin update not needed after the last step

            # ---- epilogue ----
            nc.sync.dma_start(dbg_pool, pooled.rearrange("p t g -> p (t g)"))
            logcp = wpool.tile([24, 24], dt.float32, tag="logcp")
            nc.vector.tensor_copy(logcp[:], cls_ps.rearrange("p t g -> p (t g)"))
            nc.sync.dma_start(dbg_log, logcp[:])

            sums = wpool.tile([24, 3], dt.float32, tag="sums")
            nc.vector.tensor_reduce(sums[:], sall[:], mybir.AxisListType.X,
                                    Alu.add)
            o_sb = wpool.tile([24, 3], dt.float32, tag="osb")
            nc.vector.tensor_scalar_mul(o_sb[:], sums[:], 1.0 / NTAU)

            # out[g*12+ls, o] <- o_sb[ls*2+o, g]; linear out idx = g*24 + p
            of = out_d.rearrange("a o -> (a o)")
            nc.sync.dma_start(of[0:48].rearrange("(g p) -> p g", g=2), o_sb[:, 0:2])
            nc.sync.dma_start(of[48:64].rearrange("(g p) -> p g", g=1), o_sb[0:16, 2:3])

    nc.compile()
    return nc


def _get_nc(has_bias=True):
    key = f"nc{has_bias}"
    if key not in _CACHE:
        _CACHE[key] = build_nc(has_bias=has_bias)
    return _CACHE[key]


def kernel(x, w_emb, b_emb, w_ih, w_hh, b_ih, b_hh, w_cls, b_cls):
    from concourse import bass_utils

    has_bias = bool(np.any(np.asarray(b_emb) != 0.0))
    nc = _get_nc(has_bias)
    consts = build_consts(w_emb, b_emb, w_ih, w_hh, b_ih, b_hh, w_cls, b_cls)

    x = np.asarray(x, np.float32)
    in_maps = []
    for c in range(8):
        xe = np.ascontiguousarray(x[32 * c:32 * (c + 1), 0, 1:65, :], np.float32)
        in_maps.append(dict(xe=xe, **consts))

    res = bass_utils.run_bass_kernel_spmd(nc, in_maps, core_ids=list(range(8)))
    out = np.concatenate([r["out"] for r in res.results], axis=0)
    return np.ascontiguousarray(out, np.float32)


if __name__ == "__main__":
    nc = build_nc()
    print("built ok")
